# revision 18
# baseline (speedup 1.0000x reference)
"""Causal self-attention TRN2 kernel (bf16, software-pipelined).

Problem: B=4, L=2048, D=768, H=6 heads, head_dim=128, fp32 in/out, causal
mask tril(k=1) (query q attends keys k <= q+1).

Sharding: 8 cores = 4 batches x 2 head-groups (3 heads each). Each core:
    Q = x_b @ Wq[:, cols]  (K, V likewise; biases added during PSUM->SBUF)
    per head: S^T = K @ Q^T (scaled), P = exp(S) masked, O = P@V / rowsum
    y_core = (O_heads @ Wo[rows, :])^T          -> [768, 2048] partial
Host: out[b] = (y[2b] + y[2b+1])^T + bo + bv @ Wo.

v2 design notes:
- All matmul operands bf16 (host pre-casts x and weights); PSUM fp32.
  End-to-end rel err ~4e-3 (gate 2e-2); halves DMA bytes; doubles DVE
  throughput on SBUF elementwise ops.
- Softmax denominator accumulated on DVE (esum += exp tile) instead of a
  third PE matmul stream per block; one ones-matmul per chunk-head reduces
  esum across partitions + broadcasts. Removes ~20% of PE work.
- exp batched over pairs of full K-blocks ([128,1024] PSUM across 2 banks)
  to amortize ACT's per-op access overhead.
- Causal mask multiply narrowed to the 128-wide column window per band
  block (right of it all-keep, left of band start skipped).
- DMAs batched into single strided descriptors (14 total vs 75) to unclog
  the 625ns-per-DMA HWDGE issue path at kernel start.
- Band blocks' S outputs are packed side by side into two grouped PSUM
  tiles per chunk-head so the whole diagonal band needs only 2 exp ops.
- Schedule: proj emits K first, then Q, then V; attention of chunk c
  interleaves with projection of chunk c+1 via a dependency-aware driver
  (attn units declare needed Q/K/V labels; the driver advances the proj
  generator until they are emitted). Output projections are deferred into
  later steps as PE filler for ACT-bound attention spans. PV/esum
  consumption of each exp tile is deferred pend_depth units; each head's
  denominator/normalize finalize is deferred into the next head's stream;
  outproj per-do groups are software-pipelined (h0/h1 matmuls run ahead).
TimelineSim: 168.0us (prior f32r baseline) -> 128.5us.

v3 (steady-state/marginal tuning): the graded HW time is the per-rep
MARGINAL of an N-rep NEFF (linear in reps out to 65). The bass global
list-scheduler overlaps each rep's tail (outproj3 + y DMAs) with the
next rep's proj0/attn0, so marginal (119.1us sim) < single-shot
(128.5us). Changes, all validated on the marginal metric in
TimelineSim:
- denb_fold=True: DVE-fold the two esum halves before ONE ones-matmul
  per chunk-head (halves denominator PE work, ~1us).
- Interleave pacing retuned for the marginal regime (n_attn
  [13,18,40,55], pend_depth 4, est_bufs 10, vt_eng mixed2 via random
  search): pulls proj/outproj PE filler deeper into the attn2/attn3
  exp-walls where rep-boundary mixing starves PE (~4us).
- Tried and rejected (worse in sim): rot/merge schedules, tail
  outproj-split (h0/h1 early to SBUF + h2-only tail), kv double-buffer
  across reps (PE queue is in-order so WAR isn't binding), engine
  rebalance of PSUM->SBUF copies to DVE/Pool (Pool has no PSUM access;
  DVE sits on the fin critical chain).
TimelineSim marginal: 119.1us -> 113.9us. HW ~1.27x sim (engine
microbenches all run FASTER than the cost model on HW: mm 0.88, exp
0.86, DVE 0.93, chains 0.92-1.13; the residual is unattributed
sync/contention overhead uniform enough that sim deltas transfer
directionally). HW timing noise: tunnel dispatch ~82-90ms with
heavy-tailed jitter and per-runner floor wobble ~±0.3ms; only min-based
differencing over >=300 interleaved rounds with rep-span >=32 resolves
kernel deltas, and then only to ~±4us.
"""

import math
from contextlib import ExitStack

import ml_dtypes
import numpy as np

import concourse.tile as tile
from concourse import bacc, mybir
from concourse.bass_utils import run_bass_kernel_spmd

F32 = mybir.dt.float32
BF16 = mybir.dt.bfloat16
AF = mybir.ActivationFunctionType

B, L, D, H = 4, 2048, 768, 6
HD = 128           # head dim
HPC = 3            # heads per core
DH = HPC * HD      # 384: per-core projection width
NCORES = 8
P = 128
CHUNK = 512        # q-chunk width
NCHUNK = L // CHUNK
LT = L // P        # 16 L-tiles
DT = D // P        # 6 d-tiles
SCALE = 1.0 / math.sqrt(HD)

_cache = {}


def _align8(v, up=False):
    return (v + 7) // 8 * 8 if up else v // 8 * 8


def build_nc(reps=1, enable_asserts=False, mask_eng="dve", qk_eng="act",
             vt_eng="mixed2", yst_eng="act", mm1_bufs=2, est_bufs=10,
             qt_bufs=2, interleave=True, y_bf16=True, pair_exp=True,
             split_first=2, op_late=True, sched="base", pend_depth=4,
             denb_fold=True, warmup=16, esum_split=False, kv_bufs=1,
             n_attn=None, tail_split=False):
    nc = bacc.Bacc(
        "TRN2",
        target_bir_lowering=False,
        debug=False,
        enable_asserts=enable_asserts,
        num_devices=NCORES,
    )
    MMDT = BF16
    YDT = BF16 if y_bf16 else F32
    x_d = nc.dram_tensor("x", [D, L], MMDT, kind="ExternalInput").ap()
    wq_d = nc.dram_tensor("wq", [D, DH], MMDT, kind="ExternalInput").ap()
    wk_d = nc.dram_tensor("wk", [D, DH], MMDT, kind="ExternalInput").ap()
    wv_d = nc.dram_tensor("wv", [D, DH], MMDT, kind="ExternalInput").ap()
    wo_d = nc.dram_tensor("wo", [DH, D], MMDT, kind="ExternalInput").ap()
    bq_d = nc.dram_tensor("bq", [DH], F32, kind="ExternalInput").ap()
    bk_d = nc.dram_tensor("bk", [DH], F32, kind="ExternalInput").ap()
    y_d = nc.dram_tensor("y", [D, L], YDT, kind="ExternalOutput").ap()

    def eng(name):
        return {"dve": nc.vector, "pool": nc.gpsimd}[name]

    with tile.TileContext(nc) as tc, ExitStack() as ctx:
        const = ctx.enter_context(tc.tile_pool(name="const", bufs=1))
        wpool = ctx.enter_context(tc.tile_pool(name="wts", bufs=1))
        kvpool = ctx.enter_context(tc.tile_pool(name="kv", bufs=kv_bufs))
        xtpool = ctx.enter_context(tc.tile_pool(name="xt", bufs=2))
        qpool = ctx.enter_context(tc.tile_pool(name="qt", bufs=qt_bufs))
        estpool = ctx.enter_context(tc.tile_pool(name="est", bufs=est_bufs))
        smpool = ctx.enter_context(tc.tile_pool(name="sm", bufs=2))
        opool = ctx.enter_context(tc.tile_pool(name="ot", bufs=2))
        espool = ctx.enter_context(tc.tile_pool(name="esum", bufs=2))
        esppool = ctx.enter_context(tc.tile_pool(name="esp", bufs=2))
        ypool = ctx.enter_context(tc.tile_pool(name="yst", bufs=2))
        yppool = ctx.enter_context(tc.tile_pool(name="yp", bufs=2))
        # PSUM: mm1 pairs 2x2 banks + acc 2 + pvacc 2 = 8 banks
        ps_mm = ctx.enter_context(tc.tile_pool(name="psmm", bufs=mm1_bufs, space="PSUM"))
        ps_acc = ctx.enter_context(tc.tile_pool(name="psacc", bufs=2, space="PSUM"))

        ones_mf = const.tile([P, P], F32, tag="ones_mf", name="ones_mf")
        nc.vector.memset(ones_mf[:], 1.0)
        ones_mat = const.tile([P, P], MMDT, tag="ones_mat", name="ones_mat")
        nc.scalar.copy(ones_mat[:], ones_mf[:])
        # 5 diagonal-band masks (0/1): mask[j][kp, qq] = 1 iff kp - qq <= 1 - 128*j
        masks = []
        for j in range(5):
            mjf = const.tile([P, CHUNK], F32, tag=f"maskf{j}", name=f"maskf{j}")
            nc.gpsimd.memset(mjf[:], 1.0)
            nc.gpsimd.affine_select(
                out=mjf[:],
                in_=mjf[:],
                pattern=[[1, CHUNK]],
                compare_op=mybir.AluOpType.is_ge,
                fill=0.0,
                base=1 - 128 * j,
                channel_multiplier=-1,
            )
            mj = const.tile([P, CHUNK], MMDT, tag=f"mask{j}", name=f"mask{j}")
            nc.scalar.copy(mj[:], mjf[:])
            masks.append(mj)

        # PE warmup: dummy matmuls during the initial DMA wait flip the HAM
        # clock gate to 2.4 GHz before the first real matmul arrives.
        if warmup:
            pwarm = ps_acc.tile([P, CHUNK], F32, tag="acc", name="pwarm")
            for _ in range(warmup):
                nc.tensor.matmul(pwarm[:, 0:P], ones_mat[:], ones_mat[:],
                                 start=True, stop=True)

        # ---- batched weight/bias DMAs (single strided descriptors) ----
        # wk/wv first: the schedule consumes K, V projections before Q.
        # The first wk/x0 transfers gate the first matmul, so they are split
        # into d-tile halves to shorten the critical start latency.
        def wdma(t, src, nsplit=1, dq=None):
            step = DT // nsplit
            for s in range(0, DT, step):
                (dq or nc.sync).dma_start(
                    out=t[:, s * DH:(s + step) * DH].rearrange(
                        "p (d c) -> p d c", d=step),
                    in_=src.rearrange("(d p) c -> p d c", p=P)[:, s:s + step],
                )

        def make_xT(c, nsplit=1, dq=None):
            xT = xtpool.tile([P, DT * CHUNK], MMDT, tag="xT", name="xT")
            step = DT // nsplit
            for s in range(0, DT, step):
                (dq or nc.sync).dma_start(
                    out=xT[:, s * CHUNK:(s + step) * CHUNK].rearrange(
                        "p (d c) -> p d c", d=step),
                    in_=x_d.rearrange("(d p) l -> p d l", p=P)[
                        :, s:s + step, c * CHUNK:(c + 1) * CHUNK],
                )
            return xT

        # start DMAs split across the SP and ACT hardware DGE queues so the
        # first projection's inputs (wk + x0) land as early as possible
        wk_t = wpool.tile([P, DT * DH], MMDT, tag="wk", name="wk")
        wdma(wk_t, wk_d, nsplit=split_first, dq=nc.sync)
        xT0 = make_xT(0, nsplit=split_first, dq=nc.scalar)
        wv_t = wpool.tile([P, DT * DH], MMDT, tag="wv", name="wv")
        wdma(wv_t, wv_d, dq=nc.scalar)
        wq_t = wpool.tile([P, DT * DH], MMDT, tag="wq", name="wq")
        wdma(wq_t, wq_d)
        bq_t = wpool.tile([P, HPC], F32, tag="bq", name="bq")
        nc.sync.dma_start(out=bq_t[:], in_=bq_d.rearrange("(h p) -> p h", p=P))
        bk_t = wpool.tile([P, HPC], F32, tag="bk", name="bk")
        nc.sync.dma_start(out=bk_t[:], in_=bk_d.rearrange("(h p) -> p h", p=P))
        # wo rides the ACT queue: it is not needed until the first outproj,
        # and keeping it off the SP queue unblocks the x1 chunk DMA
        wo_t = wpool.tile([P, HPC * D], MMDT, tag="wo", name="wo")
        nc.scalar.dma_start(
            out=wo_t[:].rearrange("p (h c) -> p h c", h=HPC),
            in_=wo_d.rearrange("(h p) c -> p h c", p=P),
        )

        def wk_ap(d, h):
            return wk_t[:, d * DH + h * P:d * DH + (h + 1) * P]

        def wq_ap(d, h):
            return wq_t[:, d * DH + h * P:d * DH + (h + 1) * P]

        def wv_ap(d):
            return wv_t[:, d * DH:(d + 1) * DH]

        def wo_ap(h, do):
            return wo_t[:, h * D + do * P:h * D + (do + 1) * P]

        # K^T per head [hd=128, L]; V per L-tile [kpos=128, 3*hd]
        # (re-allocated per rep so kv_bufs=2 rotates buffers across reps)
        kT = [None] * HPC
        vt = [None] * LT

        def alloc_kv():
            for h in range(HPC):
                kT[h] = kvpool.tile([P, L], MMDT, tag=f"kT{h}", name=f"kT{h}")
            for tt in range(LT):
                vt[tt] = kvpool.tile([P, DH], MMDT, tag=f"v{tt}", name=f"v{tt}")

        def qk_copy(dst, src, bias, which="q"):
            e = qk_eng if qk_eng != "mixed" else ("act" if which == "q" else "dve")
            if e == "act":
                nc.scalar.activation(dst, src, AF.Identity, bias=bias)
            else:
                nc.vector.tensor_scalar_add(dst, src, bias)

        qTs = {}

        def proj_units(c, xT=None):
            """Yields label per unit: K0..K2, V0..V3, Q0..Q2 (that order)."""
            if xT is None:
                xT = make_xT(c)
            for h in range(HPC):
                pk = ps_acc.tile([P, CHUNK], F32, tag="acc", name="acc")
                for d in range(DT):
                    nc.tensor.matmul(
                        pk[:], wk_ap(d, h), xT[:, d * CHUNK:(d + 1) * CHUNK],
                        start=(d == 0), stop=(d == DT - 1),
                    )
                qk_copy(kT[h][:, c * CHUNK:(c + 1) * CHUNK], pk[:],
                        bk_t[:, h:h + 1], which="k")
                yield f"K{h}@{c}"
            qT = qTs[c]
            for h in range(HPC):
                pq = ps_acc.tile([P, CHUNK], F32, tag="acc", name="acc")
                for d in range(DT):
                    nc.tensor.matmul(
                        pq[:], wq_ap(d, h), xT[:, d * CHUNK:(d + 1) * CHUNK],
                        start=(d == 0), stop=(d == DT - 1),
                    )
                qk_copy(qT[h][:], pq[:], bq_t[:, h:h + 1])
                yield f"Q{h}@{c}"
            for i in range(CHUNK // P):
                t = c * (CHUNK // P) + i
                pv = ps_acc.tile([P, DH], F32, tag="acc", name="acc")
                for d in range(DT):
                    nc.tensor.matmul(
                        pv[:],
                        xT[:, d * CHUNK + i * P: d * CHUNK + (i + 1) * P],
                        wv_ap(d),
                        start=(d == 0), stop=(d == DT - 1),
                    )
                if vt_eng == "mixed":
                    ve = "dve" if c < 2 else "act"
                elif vt_eng == "mixed2":
                    ve = "act" if c < 2 else "dve"
                else:
                    ve = vt_eng
                if ve == "act":
                    nc.scalar.copy(vt[t][:], pv[:])
                else:
                    nc.vector.tensor_copy(vt[t][:], pv[:])
                yield f"V{i}@{c}"

        def attn_units(c):
            """Yields None or ("need", [labels]) before units needing them."""
            qT = qTs[c]
            KB = 4 * c + 5 if c < NCHUNK - 1 else LT
            NFULL = 4 * c
            oTn = [opool.tile([P, CHUNK], MMDT, tag=f"oT{h}", name=f"oT{h}")
                   for h in range(HPC)]
            fin_prev = [None]
            oT_live[c] = oTn
            for h in range(HPC):
                yield ("need", [f"Q{h}@{c}"])
                po = ps_acc.tile([P, CHUNK], F32, tag="pvacc", name="pvacc", bufs=2)
                esum = espool.tile([P, 2 * CHUNK], MMDT, tag="esum", name="esum")
                # second accumulator on the idle GPSIMD engine for alternate
                # pair-adds (parallel chain, merged once at finalize)
                use_esp = esum_split and NFULL >= 8
                esumP = (esppool.tile([P, 2 * CHUNK], MMDT, tag="esp",
                                      name="esp") if use_esp else None)
                state = {"first": True, "firstP": True, "npair": 0}

                def consume(unit, _h=h, _po=po, _esum=esum, _state=state, _KB=KB):
                    kind, blocks, est = unit
                    hsl = slice(_h * P, (_h + 1) * P)
                    if kind == "pair":
                        kb = blocks
                        nc.tensor.matmul(
                            _po[:], vt[kb][:, hsl], est[:, 0:CHUNK],
                            start=(kb == 0), stop=False,
                        )
                        nc.tensor.matmul(
                            _po[:], vt[kb + 1][:, hsl], est[:, CHUNK:2 * CHUNK],
                            start=False, stop=(kb + 1 == _KB - 1),
                        )
                        _state["npair"] += 1
                        if esumP is not None and _state["npair"] % 2 == 0:
                            if _state["firstP"]:
                                nc.gpsimd.tensor_copy(esumP[:], est[:])
                                _state["firstP"] = False
                            else:
                                nc.gpsimd.tensor_add(esumP[:], esumP[:], est[:])
                        elif _state["first"]:
                            nc.vector.tensor_copy(_esum[:], est[:])
                            _state["first"] = False
                        else:
                            nc.vector.tensor_add(_esum[:], _esum[:], est[:])
                    else:  # band group: blocks = [(kb, off, s0)], esum half A
                        for kb, off, s0 in blocks:
                            j = kb - NFULL
                            w = CHUNK - s0
                            m1 = min(CHUNK, _align8(128 * j + 126, up=True))
                            if m1 > s0:
                                eng(mask_eng).tensor_mul(
                                    est[:, off:off + (m1 - s0)],
                                    est[:, off:off + (m1 - s0)],
                                    masks[j][:, s0:m1],
                                )
                            nc.tensor.matmul(
                                _po[:, s0:CHUNK], vt[kb][:, hsl],
                                est[:, off:off + w],
                                start=(kb == 0), stop=(kb == _KB - 1),
                            )
                            if _state["first"]:
                                nc.vector.tensor_copy(
                                    _esum[:, s0:CHUNK], est[:, off:off + w])
                                _state["first"] = False
                            else:
                                nc.vector.tensor_add(
                                    _esum[:, s0:CHUNK], _esum[:, s0:CHUNK],
                                    est[:, off:off + w],
                                )

                pending = []
                # full-block pairs (NFULL is always even)
                for m in range(0, NFULL, 2):
                    pst = ps_mm.tile([P, 2 * CHUNK], F32, tag="mm1", name="mm1")
                    nc.tensor.matmul(
                        pst[:, 0:CHUNK], kT[h][:, m * P:(m + 1) * P], qT[h][:],
                        start=True, stop=True,
                    )
                    nc.tensor.matmul(
                        pst[:, CHUNK:2 * CHUNK],
                        kT[h][:, (m + 1) * P:(m + 2) * P], qT[h][:],
                        start=True, stop=True,
                    )
                    est = estpool.tile([P, 2 * CHUNK], MMDT, tag="est", name="est")
                    if pair_exp:
                        nc.scalar.activation(est[:], pst[:], AF.Exp, scale=SCALE)
                    else:
                        nc.scalar.activation(est[:, 0:CHUNK], pst[:, 0:CHUNK],
                                             AF.Exp, scale=SCALE)
                        nc.scalar.activation(est[:, CHUNK:], pst[:, CHUNK:],
                                             AF.Exp, scale=SCALE)
                    pending.append(("pair", m, est))
                    if len(pending) > pend_depth:
                        consume(pending.pop(0))
                    if fin_prev[0] is not None:
                        fin_prev[0]()
                        fin_prev[0] = None
                    yield None
                # diagonal band blocks, packed into 2 grouped exp tiles:
                # group 1 = [j0, j1], group 2 = [j2, j3(, j4)] with each
                # block's S output placed side by side in one PSUM tile
                band = list(range(NFULL, KB))
                groups = [band[:2], band[2:]]
                for grp in groups:
                    if not grp:
                        continue
                    needs = []
                    blocks = []
                    off = 0
                    for kb in grp:
                        j = kb - NFULL
                        needs += [f"K{h}@{kb // 4}", f"V{kb % 4}@{kb // 4}"]
                        s0 = _align8(max(0, 128 * j - 2))
                        blocks.append((kb, off, s0))
                        off += CHUNK - s0
                    yield ("need", needs)
                    pst = ps_mm.tile([P, 2 * CHUNK], F32, tag="mm1", name="mm1")
                    for kb, o, s0 in blocks:
                        nc.tensor.matmul(
                            pst[:, o:o + CHUNK - s0],
                            kT[h][:, kb * P:(kb + 1) * P], qT[h][:, s0:CHUNK],
                            start=True, stop=True,
                        )
                    est = estpool.tile([P, 2 * CHUNK], MMDT, tag="est", name="est")
                    nc.scalar.activation(est[:, 0:off], pst[:, 0:off],
                                         AF.Exp, scale=SCALE)
                    pending.append(("band", blocks, est))
                    if len(pending) > pend_depth:
                        consume(pending.pop(0))
                    if fin_prev[0] is not None:
                        fin_prev[0]()
                        fin_prev[0] = None
                    yield None
                for u in pending:
                    consume(u)
                if fin_prev[0] is not None:
                    fin_prev[0]()
                    fin_prev[0] = None
                # denominator: reduce esum across partitions, broadcast,
                # recip, normalize. Deferred into the next head's stream so
                # its serial chain hides behind fresh S/exp work.
                def fin(_h=h, _po=po, _esum=esum, _esumP=esumP, _state=state):
                    if _esumP is not None and not _state["firstP"]:
                        nc.vector.tensor_add(_esum[:], _esum[:], _esumP[:])
                    pd = ps_mm.tile([P, 2 * CHUNK], F32, tag="mm1", name="mm1")
                    if denb_fold and NFULL > 0:
                        nc.vector.tensor_add(_esum[:, 0:CHUNK],
                                             _esum[:, 0:CHUNK],
                                             _esum[:, CHUNK:2 * CHUNK])
                        nc.tensor.matmul(pd[:, 0:CHUNK], ones_mat[:],
                                         _esum[:, 0:CHUNK],
                                         start=True, stop=True)
                    else:
                        nc.tensor.matmul(pd[:, 0:CHUNK], ones_mat[:],
                                         _esum[:, 0:CHUNK],
                                         start=True, stop=(NFULL == 0))
                        if NFULL > 0:
                            nc.tensor.matmul(pd[:, 0:CHUNK], ones_mat[:],
                                             _esum[:, CHUNK:2 * CHUNK],
                                             start=False, stop=True)
                    recip = smpool.tile([P, CHUNK], F32, tag="recip",
                                        name="recip")
                    nc.vector.reciprocal(recip[:], pd[:, 0:CHUNK])
                    nc.vector.tensor_mul(oTn[_h][:], _po[:], recip[:])
                    fins_done.add((c, _h))

                if h == HPC - 1:
                    fin()
                else:
                    fin_prev[0] = fin
                yield None
            oT_out[c] = oTn

        oT_out = {}
        oT_live = {}
        yp_out = {}
        fins_done = set()

        def outproj_early(c):
            """h0+h1 partial outproj into an SBUF tile, emitted as PE filler
            inside attn(c) once heads 0 and 1 are finalized ("stall" marks
            units whose inputs aren't emitted yet)."""
            while (c, 0) not in fins_done or (c, 1) not in fins_done:
                yield "stall"
            oTn = oT_live[c]
            yp = yppool.tile([P, DT * CHUNK], YDT, tag="yp", name="yp")
            yp_out[c] = yp
            for do in range(DT):
                py = ps_acc.tile([P, CHUNK], F32, tag="acc", name="acc")
                for h in range(HPC - 1):
                    nc.tensor.matmul(py[:], wo_ap(h, do), oTn[h][:],
                                     start=(h == 0), stop=(h == HPC - 2))
                if tail_split == "dve":
                    nc.vector.tensor_copy(
                        yp[:, do * CHUNK:(do + 1) * CHUNK], py[:])
                else:
                    nc.scalar.copy(yp[:, do * CHUNK:(do + 1) * CHUNK], py[:])
                yield f"OE{do}@{c}"

        def outproj_late(c):
            """h2 matmul + add of the h01 partial + store; the only
            post-attention tail work for chunk c."""
            oTn = oT_out.pop(c)
            yp = yp_out.pop(c)
            yst = ypool.tile([P, DT * CHUNK], YDT, tag="yst", name="yst")

            def finish(do, py):
                nc.vector.tensor_add(
                    yst[:, do * CHUNK:(do + 1) * CHUNK], py[:],
                    yp[:, do * CHUNK:(do + 1) * CHUNK])
                if do % 2 == 1:
                    s = do - 1
                    nc.sync.dma_start(
                        out=y_d.rearrange("(d p) l -> p d l", p=P)[
                            :, s:do + 1, c * CHUNK:(c + 1) * CHUNK],
                        in_=yst[:, s * CHUNK:(do + 1) * CHUNK].rearrange(
                            "p (d c) -> p d c", d=do + 1 - s),
                    )

            pend = None
            for do in range(DT):
                py = ps_acc.tile([P, CHUNK], F32, tag="acc", name="acc")
                nc.tensor.matmul(py[:], wo_ap(HPC - 1, do), oTn[HPC - 1][:],
                                 start=True, stop=True)
                if pend is not None:
                    finish(*pend)
                pend = (do, py)
                yield f"O{do}@{c}"
            finish(*pend)

        def outproj_units(c):
            oTn = oT_out.pop(c)
            yst = ypool.tile([P, DT * CHUNK], YDT, tag="yst", name="yst")
            ye = yst_eng if yst_eng != "mixed" else ("act" if c < 1 else "dve")

            def finish(do, py):
                nc.tensor.matmul(py[:], wo_ap(HPC - 1, do), oTn[HPC - 1][:],
                                 start=False, stop=True)
                dst = yst[:, do * CHUNK:(do + 1) * CHUNK]
                if ye == "act":
                    nc.scalar.copy(dst, py[:])
                else:
                    nc.vector.tensor_copy(dst, py[:])
                if do % 2 == 1:
                    s = do - 1
                    nc.sync.dma_start(
                        out=y_d.rearrange("(d p) l -> p d l", p=P)[
                            :, s:do + 1, c * CHUNK:(c + 1) * CHUNK],
                        in_=yst[:, s * CHUNK:(do + 1) * CHUNK].rearrange(
                            "p (d c) -> p d c", d=do + 1 - s),
                    )

            pend = None
            for do in range(DT):
                py = ps_acc.tile([P, CHUNK], F32, tag="acc", name="acc")
                for h in range(HPC - 1):
                    nc.tensor.matmul(py[:], wo_ap(h, do), oTn[h][:],
                                     start=(h == 0), stop=False)
                if pend is not None:
                    finish(*pend)
                pend = (do, py)
                yield f"O{do}@{c}"
            finish(*pend)

        seen = set()

        def drain(g):
            for lbl in g:
                if lbl is not None and not (isinstance(lbl, tuple)):
                    seen.add(lbl)

        def chain(*gens):
            for g in gens:
                yield from g

        def ileave(a, b, na, nb):
            """Interleave a (attn) with b (labeled); honor a's need markers."""
            err = 0
            alive_a = alive_b = True
            while alive_a or alive_b:
                take_a = alive_a and (err < na or not alive_b)
                if take_a:
                    try:
                        item = next(a)
                        err += nb
                        while isinstance(item, tuple) and item[0] == "need":
                            missing = [x for x in item[1] if x not in seen]
                            while missing and alive_b:
                                try:
                                    seen.add(next(b))
                                except StopIteration:
                                    alive_b = False
                                    break
                                missing = [x for x in item[1] if x not in seen]
                            item = next(a)
                    except StopIteration:
                        alive_a = False
                elif alive_b:
                    try:
                        seen.add(next(b))
                        err -= na
                    except StopIteration:
                        alive_b = False

        if n_attn is None:
            n_attn = [13, 18, 40, 55]
        for _rep in range(reps):
            alloc_kv()
            fins_done.clear()
            for c in range(NCHUNK):
                qTs[c] = [qpool.tile([P, CHUNK], MMDT, tag=f"qT{h}",
                                     name=f"qT{h}") for h in range(HPC)]
            if interleave and sched == "merge":
                # single continuous interleave: attention stream vs
                # proj/outproj stream, paced by the needs-driver
                drain(proj_units(0, xT=xT0 if _rep == 0 else None))
                ileave(chain(attn_units(0), attn_units(1), attn_units(2),
                             attn_units(3)),
                       chain(proj_units(1), proj_units(2), proj_units(3),
                             outproj_units(0), outproj_units(1),
                             outproj_units(2)),
                       105, 48)
                drain(outproj_units(3))
            elif interleave and sched == "rot":
                # attention order 1, 3, 2, 0: heaviest attention (c=3) sits
                # mid-kernel where proj3/outproj PE work hides its exp wall;
                # the lightest (c=0) forms a short PE-bound tail.
                drain(proj_units(0, xT=xT0 if _rep == 0 else None))
                ileave(attn_units(1),
                       chain(proj_units(1), proj_units(2)), n_attn[1], 20)
                ileave(attn_units(3),
                       chain(proj_units(3), outproj_units(1)), n_attn[3], 16)
                ileave(attn_units(2), outproj_units(3), n_attn[2], 6)
                ileave(attn_units(0), outproj_units(2), n_attn[0], 6)
                drain(outproj_units(0))
            elif interleave:
                drain(proj_units(0, xT=xT0 if _rep == 0 else None))
                ileave(attn_units(0), proj_units(1), n_attn[0], 10)
                if op_late and tail_split:
                    ileave(attn_units(1), proj_units(2), n_attn[1], 10)
                    ileave(attn_units(2),
                           chain(proj_units(3), outproj_units(0)), n_attn[2], 16)
                    ileave(attn_units(3),
                           chain(outproj_units(1), outproj_units(2),
                                 outproj_early(3)),
                           n_attn[3], 12)
                    drain(outproj_late(3))
                elif op_late:
                    ileave(attn_units(1), proj_units(2), n_attn[1], 10)
                    ileave(attn_units(2),
                           chain(proj_units(3), outproj_units(0)), n_attn[2], 16)
                    ileave(attn_units(3),
                           chain(outproj_units(1), outproj_units(2)),
                           n_attn[3], 12)
                else:
                    ileave(attn_units(1),
                           chain(proj_units(2), outproj_units(0)), n_attn[1], 16)
                    ileave(attn_units(2),
                           chain(proj_units(3), outproj_units(1)), n_attn[2], 16)
                    ileave(attn_units(3), outproj_units(2), n_attn[3], 6)
                if not (op_late and tail_split):
                    drain(outproj_units(3))
            else:
                drain(proj_units(0, xT=xT0 if _rep == 0 else None))
                drain(proj_units(1))
                drain(attn_units(0))
                drain(proj_units(2))
                drain(outproj_units(0))
                drain(attn_units(1))
                drain(proj_units(3))
                drain(outproj_units(1))
                drain(attn_units(2))
                drain(outproj_units(2))
                drain(attn_units(3))
                drain(outproj_units(3))

    nc.compile()
    return nc


def shard_inputs(x, Wq, bq, Wk, bk, Wv, bv, Wo, bo):
    bf = ml_dtypes.bfloat16
    x = np.asarray(x, dtype=np.float32)
    in_maps = []
    for core in range(NCORES):
        b = core // 2
        g = core % 2
        sl = slice(g * DH, (g + 1) * DH)
        in_maps.append({
            "x": np.ascontiguousarray(x[b].T.astype(bf)),
            "wq": np.ascontiguousarray(np.asarray(Wq, np.float32)[:, sl].astype(bf)),
            "wk": np.ascontiguousarray(np.asarray(Wk, np.float32)[:, sl].astype(bf)),
            "wv": np.ascontiguousarray(np.asarray(Wv, np.float32)[:, sl].astype(bf)),
            "wo": np.ascontiguousarray(np.asarray(Wo, np.float32)[sl, :].astype(bf)),
            "bq": np.ascontiguousarray(np.asarray(bq, np.float32)[sl]),
            "bk": np.ascontiguousarray(np.asarray(bk, np.float32)[sl]),
        })
    return in_maps


def unshard_output(results, Wo, bv, bo):
    out = np.empty((B, L, D), dtype=np.float32)
    for b in range(B):
        acc = (results[2 * b]["y"].astype(np.float32)
               + results[2 * b + 1]["y"].astype(np.float32))  # [D, L]
        out[b] = acc.T
    corr = np.asarray(bo, np.float32) + np.asarray(bv, np.float32) @ np.asarray(
        Wo, np.float32
    )
    out += corr
    return out


def run(inputs, trace=False, **kw):
    if "nc" not in _cache:
        _cache["nc"] = build_nc()
    nc = _cache["nc"]
    in_maps = shard_inputs(**inputs)
    res = run_bass_kernel_spmd(nc, in_maps, list(range(NCORES)), trace=trace, **kw)
    out = unshard_output(res.results, inputs["Wo"], inputs["bv"], inputs["bo"])
    return out, res


def kernel(**inputs):
    out, _ = run(inputs)
    return out

